# revision 15
# baseline (speedup 1.0000x reference)
"""ClusterKLLoss Trainium2 kernel (8 NeuronCores, 2D-sharded: 2 i-halves x 4
j-quarters).

Math (derived from the reference):
  loss = CE(logits, arange(B), sum) / B  with logits[i,j] = -kl[i,j]/T
  kl[i,j] = hneg[j] - Li[i] . Q[j],  Q = softmax(c_j), hneg[j] = sum Q log Q.
  Per-row (i) constant shifts cancel in log-softmax, so log_softmax(c_i) is
  never needed:
    G[i,j] = (c_i[i] . Q[j] - hneg[j]) / T       (logits up to per-row shift)
  With E = exp(c_j), Z_j = sum E[j], A_j = sum E[j]*c_j[j]:
    S[i,j] = c_i[i] . W'[j] + b'_j = G[i,j]*C,   W' = E*C/(T*Z_j),
    b'_j = (ln Z_j)*(C/T) - A_j*(C/(T*Z_j))
  loss = sum_i (ln sum_j exp(S[i,j]/C) - S[i,i]/C) / B

Sharding: core c = (h, p), h = c//4, p = c%4. Core holds c_i rows
[2048h, 2048h+2048) and c_j rows [2048h+1024p, +1024) mod B. The four cores
of a half together cover all j, so the host sums their per-row partial
softmax sums and takes the log. The diagonal S[i,i] lands in cores p=0
(m-tiles 0-7) and p=1 (m-tiles 8-15); every core extracts the same local
window (cols 128*(m%8)) and the host keeps the valid ones.

Schedule notes: the A reduces are deferred into the matmul phase (prod
tiles stay alive), ws transposes alternate between the sync and scalar
HWDGE rings with lag-1 emission, ciT is a rotating 5-slot pool (not fully
resident) so the freed SBUF buys deep ci load buffering, and ci loads ride
the gpsimd SWDGE ring sequenced after the cj loads.

Per-core outputs: out[:, 0:32] = Zi (partial sum_j exp(S/C), col 2m+g),
out[:, 32:48] = Dc (local diag of S).
"""

import sys

for _p in ("/opt/trn_rl_repo",):
    if _p not in sys.path:
        sys.path.insert(0, _p)

import numpy as np

import concourse.bass as bass
import concourse.bacc as bacc
import concourse.tile as tile
from concourse import mybir
from concourse import bass_utils
from concourse.bass import _add_dep_helper

B = 4096
D = 2048
TEMP = 0.5
NCORES = 8
IH = 2      # i halves
JQ = 4      # j quarters
ISH = B // IH   # 2048 rows of c_i per core
JSH = B // JQ   # 1024 rows of c_j per core
MT = ISH // 128  # 16 i-tiles
JT = JSH // 128  # 8 j-tiles
KT = D // 128    # 16 k partition-tiles
F32 = mybir.dt.float32
F16 = mybir.dt.float16
AF = mybir.ActivationFunctionType
OP = mybir.AluOpType
AX = mybir.AxisListType

CSCALE = 4096.0  # power-of-two normalizer keeping W' = E*C/(T*Z) in fp16 range
INV_C = 1.0 / CSCALE


def build_kernel_body(tc, out_ap, ci_ap, cj_ap, eye_ap):
    """out: [128,48] f32; ci: [2048,2048] f32; cj: [1024,2048] f32;
    eye: [128,128] f32."""
    nc = tc.nc

    from contextlib import ExitStack

    with ExitStack() as ctx:
        singles = ctx.enter_context(tc.tile_pool(name="singles", bufs=1))
        cjx = ctx.enter_context(tc.tile_pool(name="cjx", bufs=3))
        cix = ctx.enter_context(tc.tile_pool(name="cix", bufs=4))
        epool = ctx.enter_context(tc.tile_pool(name="epool", bufs=3))
        spool = ctx.enter_context(tc.tile_pool(name="spool", bufs=3))
        dpool = ctx.enter_context(tc.tile_pool(name="dpool", bufs=2))
        psS = ctx.enter_context(tc.tile_pool(name="psS", bufs=6, space="PSUM"))
        psX = ctx.enter_context(tc.tile_pool(name="psX", bufs=2, space="PSUM"))

        # constants
        eye32 = singles.tile([128, 128], F32)
        nc.sync.dma_start(out=eye32, in_=eye_ap)
        eye16 = singles.tile([128, 128], F16)
        nc.vector.tensor_copy(out=eye16, in_=eye32)
        ones2 = singles.tile([2, 128], F16)
        nc.vector.memset(ones2, 1.0)

        # per-j scalars, one col per local cj tile
        Zc = singles.tile([128, JT], F32)
        Ac = singles.tile([128, JT], F32)    # raw A_j = sum E*x
        sCc = singles.tile([128, JT], F32)
        # outputs
        Zi = singles.tile([128, 2 * MT], F32)
        Dc = singles.tile([128, MT], F32)
        # operand stores
        WT = singles.tile([128, KT, JT, 128], F16)   # [k, kt, jt, j]
        ciT = singles.tile([128, MT, KT, 128], F16)  # [k, m, kt, i]
        biasr = singles.tile([2, JSH], F16)          # bias hi/lo rows

        # ---- c_j preprocessing: software-pipelined across 8 tiles ----
        xts = [None] * JT
        wss = [None] * JT
        prods = [None] * JT
        cj_loads = [None] * JT

        def emit_cj_load(t):
            xt = cjx.tile([128, D], F32, tag="cjload")
            cj_loads[t] = nc.sync.dma_start(
                out=xt, in_=cj_ap[128 * t : 128 * (t + 1), :]
            )
            xts[t] = xt

        def emit_exp(t):
            es = epool.tile([128, D], F16, tag="es", bufs=2)
            nc.scalar.activation(
                out=es, in_=xts[t], func=AF.Exp, accum_out=Zc[:, t : t + 1]
            )
            # sC_j = C/(T*Z_j); W' = E*sC in fp16
            nc.vector.tensor_scalar_mul(
                sCc[:, t : t + 1], Zc[:, t : t + 1], float(TEMP / CSCALE)
            )
            nc.vector.reciprocal(out=sCc[:, t : t + 1], in_=sCc[:, t : t + 1])
            ws = epool.tile([128, D], F16, tag="ws")
            nc.vector.tensor_scalar_mul(ws, es, sCc[:, t : t + 1])
            wss[t] = ws
            prod = epool.tile([128, D], F16, tag="prod", bufs=3)
            nc.vector.tensor_mul(prod, es, xts[t])
            prods[t] = prod

        def emit_wsxpose(t):
            eng = nc.sync if t % 2 == 0 else nc.scalar
            eng.dma_start_transpose(out=WT[:, :, t, :], in_=wss[t])

        def emit_reduce(t):
            # A reduces are deferred into the matmul phase; split across
            # DVE and ACT so neither engine stalls the bias row for long
            if t % 2 == 0:
                nc.vector.tensor_reduce(
                    out=Ac[:, t : t + 1], in_=prods[t], axis=AX.X, op=OP.add
                )
            else:
                dmp = dpool.tile([128, D], F16, tag="reddump", bufs=1)
                nc.scalar.activation(
                    out=dmp,
                    in_=prods[t],
                    func=AF.Copy,
                    accum_out=Ac[:, t : t + 1],
                )

        for t in range(JT):
            emit_cj_load(t)
        for t in range(JT):
            emit_exp(t)
            if t >= 1:
                emit_wsxpose(t - 1)
            if t >= 2:
                emit_reduce(t - 2)
        emit_wsxpose(JT - 1)
        emit_reduce(JT - 2)
        emit_reduce(JT - 1)

        # ln(Z) right after the last exp (one table switch to Ln here,
        # back to Exp for the softmax phase)
        lnz = spool.tile([128, JT], F32, tag="lnz")
        nc.scalar.activation(out=lnz, in_=Zc, func=AF.Ln)

        # ---- c_i loads: gpsimd SWDGE ring, sequenced after cj loads ----
        cits = [None] * MT
        c16s = [None] * MT

        def emit_ci_load(m):
            cit = cix.tile([128, D], F32, tag="ciload")
            ld = nc.gpsimd.dma_start(
                out=cit, in_=ci_ap[128 * m : 128 * (m + 1), :]
            )
            if m == 0:
                _add_dep_helper(
                    ld.ins,
                    cj_loads[JT - 1].ins,
                    sync=True,
                    reason="ci loads after cj loads (HBM priority)",
                )
            cits[m] = cit

        def emit_ci_cast(m):
            c16 = epool.tile([128, D], F16, tag="c16", bufs=2)
            nc.vector.tensor_copy(out=c16, in_=cits[m])
            c16s[m] = c16

        def emit_ci_xpose(m):
            eng = nc.sync if m % 2 == 0 else nc.scalar
            eng.dma_start_transpose(out=ciT[:, m], in_=c16s[m])

        for m in range(MT):
            emit_ci_load(m)

        # ---- bias row: b' = lnZ*(C/T) - A*sC, split into fp16 hi+lo ----
        bp = spool.tile([128, JT], F32, tag="bp")
        nc.vector.tensor_scalar_mul(bp, lnz, float(CSCALE / TEMP))
        asc = spool.tile([128, JT], F32, tag="asc")
        nc.vector.tensor_mul(asc, Ac, sCc)
        nc.vector.tensor_sub(bp, bp, asc)
        e2 = spool.tile([128, JT, 2], F16, tag="e2")
        nc.vector.tensor_copy(out=e2[:, :, 0], in_=bp)
        nc.vector.tensor_sub(e2[:, :, 1], bp, e2[:, :, 0])
        for q in range(JT):
            e2q_ps = psX.tile([2, 128], F16, tag="xp")
            nc.tensor.transpose(e2q_ps, e2[:, q, :], eye16)
            nc.vector.tensor_copy(
                out=biasr[:, 128 * q : 128 * (q + 1)], in_=e2q_ps
            )

        for m in range(MT):
            emit_ci_cast(m)
            emit_ci_xpose(m)

        # ---- main matmuls + row softmax-exp accumulation ----
        for m in range(MT):
            for g in range(2):
                S_ps = psS.tile([128, JSH // 2], F32, tag="s")
                for kt in range(KT):
                    nc.tensor.matmul(
                        S_ps,
                        ciT[:, m, kt, :],
                        WT[:, kt, 4 * g : 4 * (g + 1), :],
                        start=(kt == 0),
                        stop=False,
                    )
                nc.tensor.matmul(
                    S_ps,
                    ones2,
                    biasr[:, 512 * g : 512 * (g + 1)],
                    start=False,
                    stop=True,
                )
                # local diag window (valid on p=0 cores for m<8, p=1 for m>=8)
                c0 = 128 * (m % 8)
                if c0 // 512 == g:
                    cg = c0 - 512 * g
                    junk = spool.tile([128, 128], F32, tag="junk")
                    nc.vector.tensor_mul(junk, S_ps[:, cg : cg + 128], eye32)
                    nc.vector.tensor_reduce(
                        out=Dc[:, m : m + 1], in_=junk, axis=AX.X, op=OP.add
                    )
                expj = dpool.tile([128, JSH // 2], F16, tag="expj")
                nc.scalar.activation(
                    out=expj,
                    in_=S_ps,
                    func=AF.Exp,
                    scale=float(INV_C),
                    accum_out=Zi[:, 2 * m + g : 2 * m + g + 1],
                )

        res = spool.tile([128, 3 * MT], F32, tag="res")
        nc.vector.tensor_copy(out=res[:, 0 : 2 * MT], in_=Zi)
        nc.vector.tensor_copy(out=res[:, 2 * MT : 3 * MT], in_=Dc)
        nc.sync.dma_start(out=out_ap, in_=res)


_NC_CACHE = {}


def build_nc():
    key = "nc_v3"
    if key in _NC_CACHE:
        return _NC_CACHE[key]
    nc = bacc.Bacc("TRN2", target_bir_lowering=False, debug=False)
    ci = nc.dram_tensor("ci", [ISH, D], F32, kind="ExternalInput").ap()
    cj = nc.dram_tensor("cj", [JSH, D], F32, kind="ExternalInput").ap()
    eye = nc.dram_tensor("eye", [128, 128], F32, kind="ExternalInput").ap()
    out = nc.dram_tensor("out", [128, 3 * MT], F32, kind="ExternalOutput").ap()
    with tile.TileContext(nc) as tc:
        build_kernel_body(tc, out, ci, cj, eye)
    nc.compile()
    _NC_CACHE[key] = nc
    return nc


def make_in_maps(c_i, c_j):
    eye = np.eye(128, dtype=np.float32)
    in_maps = []
    for c in range(NCORES):
        h, p = c // JQ, c % JQ
        sj = (ISH * h + JSH * p) % B
        in_maps.append(
            {
                "ci": np.ascontiguousarray(c_i[ISH * h : ISH * (h + 1)]),
                "cj": np.ascontiguousarray(c_j[sj : sj + JSH]),
                "eye": eye,
            }
        )
    return in_maps


def kernel(c_i, c_j, **kwargs):
    c_i = np.ascontiguousarray(np.asarray(c_i, dtype=np.float32))
    c_j = np.ascontiguousarray(np.asarray(c_j, dtype=np.float32))
    nc = build_nc()
    in_maps = make_in_maps(c_i, c_j)
    res = bass_utils.run_bass_kernel_spmd(
        nc, in_maps, core_ids=list(range(NCORES))
    )
    outs = [np.asarray(r["out"], dtype=np.float64) for r in res.results]
    loss = 0.0
    for h in range(IH):
        Zi_tot = sum(
            outs[JQ * h + p][:, 0 : 2 * MT : 2] + outs[JQ * h + p][:, 1 : 2 * MT : 2]
            for p in range(JQ)
        )
        lse = np.log(Zi_tot)
        diag = np.concatenate(
            [
                outs[JQ * h + 0][:, 2 * MT : 2 * MT + 8],
                outs[JQ * h + 1][:, 2 * MT + 8 : 2 * MT + 16],
            ],
            axis=1,
        )
        loss += (lse - diag * INV_C).sum()
    return np.float32(loss / B).reshape(())
